# revision 22
# baseline (speedup 1.0000x reference)
"""DebiasedPosLossV2 on 8 NeuronCores — fp8 produce + dual-engine exp + fp8 DoubleRow.

Decomposition: each core owns a 1024-column strip of the 8192x8192 sim
matrix; produce matmuls build z = x @ x_strip^T 128 rows at a time, exp
runs split across ScalarE (activation) and DVE (Schraudolph int8 bit
trick), and a one-hot DoubleRow consume matmul reduces rows into
per-class sums (row 0 = all-ones = full). Host finishes the loss.

Changes vs the fp16 column-strip baseline (67.3us -> ~63us):
  * x is fp8e4 end-to-end: produce matmuls run at the same 1 col/cycle
    as fp16 (fp8 without a perf mode == bf16 speed on TRN2) but the xt
    DMA halves to 1 MB, so the PE pipeline starts ~1.7us after the DMA
    queue opens instead of ~6us in.
  * no dummy-matmul HAM warmup: real produces start as soon as boot
    (4 row units + chunk-0 rhs) lands and warm the PE clock themselves
    while the rest of xt streams; xta rides the Pool engine's DMA queue
    in parallel with the Sync queue.
  * cmask is fp16, fs output is f16 DMA'd straight from mk row 0 / a
    small stile copy (no staging tile, shorter extract tail).
  * exp split 18 ScalarE / 14 DVE per chunk (ScalarE is ~15% faster and
    DVE also carries the extract work).
"""

import sys

if "/opt/trn_rl_repo" not in sys.path:
    sys.path.insert(0, "/opt/trn_rl_repo")

from contextlib import ExitStack

import numpy as np

import concourse.bass as bass
import concourse.mybir as mybir
import concourse.tile as tile
from concourse.bass import ds, ts
from concourse.bass_utils import run_bass_kernel_spmd

B = 4096
D = 128
TWO_B = 2 * B
TEMPERATURE = 0.5
TAU_PLUS = 0.1
N_CORES = 8
COLS_PER_CORE = TWO_B // N_CORES  # 1024
CHUNK = 512
N_CHUNKS = COLS_PER_CORE // CHUNK  # 2
N_PAIR = TWO_B // 256             # 32 unit-pairs per chunk
NCLS = 100
OHW = 104                         # one-hot rows: 0 ones, 1..101 classes, pad
OHP = 112                         # padded pair stride (16B aligned)

F16 = mybir.dt.float16
F32 = mybir.dt.float32
F8 = mybir.dt.float8e4
I8 = mybir.dt.int8
MULT = mybir.AluOpType.mult
ADD = mybir.AluOpType.add
DR = mybir.MatmulPerfMode.DoubleRow

# Schraudolph fp8e4 constants: bits8(e^y) ~= y*8*log2(e) + (7*8 + c)
SCH_A = (1.0 / TEMPERATURE) * 8.0 * 1.4426950408889634
SCH_B_DEFAULT = 56.0 - 0.46

# 18 of 32 pairs on ScalarE, 14 on DVE — ScalarE exp is ~15% faster and the
# DVE also carries the extract work
ACT_PAT = [((g + 1) * 18) // 32 != (g * 18) // 32 for g in range(32)]
# the ramp puts pairs 30+31 both on ScalarE — their exps then serialize in
# the pipeline drain. Swap 29<->30 so the final two exps run on different
# engines concurrently.
ACT_PAT[29], ACT_PAT[30] = True, False

_PROGRAM = None


def _build_program() -> bass.Bass:
    nc = bass.Bass()

    # DMA order tuned so the pipeline never starves: boot first (4 xt row
    # units + chunk-0 rhs), xta in parallel on the Pool queue, then schb /
    # one-hots / the rest of the xt stream ahead of their consumers.
    boot_d = nc.declare_dram_parameter("boot", [128, 512 + CHUNK], F8, isOutput=False)
    xta_d = nc.declare_dram_parameter("xta", [128, 2048], F8, isOutput=False)
    oha_d = nc.declare_dram_parameter("oha", [128, 8 * 2 * OHP], F8, isOutput=False)
    xtc1_d = nc.declare_dram_parameter("xtc1", [128, CHUNK], F8, isOutput=False)
    ohb_d = nc.declare_dram_parameter("ohb", [128, (N_PAIR - 8) * 2 * OHP], F8, isOutput=False)
    xtb_d = nc.declare_dram_parameter("xtb", [128, TWO_B - 512 - 2048], F8, isOutput=False)
    cm_d = nc.declare_dram_parameter("cmask", [NCLS + 1, COLS_PER_CORE], F16, isOutput=False)
    schb_d = nc.declare_dram_parameter("schb", [128, 1], F32, isOutput=False)
    fs_d = nc.declare_dram_parameter("fs", [1, 2 * COLS_PER_CORE], F16, isOutput=True)

    with ExitStack() as ctx:
        tc = ctx.enter_context(tile.TileContext(nc))
        const = ctx.enter_context(tc.tile_pool(name="const", bufs=1))
        mkp = ctx.enter_context(tc.tile_pool(name="mk", bufs=2))
        zp = ctx.enter_context(tc.tile_pool(name="z", bufs=3, space="PSUM"))
        qp = ctx.enter_context(tc.tile_pool(name="q", bufs=2, space="PSUM"))

        # wsrc zeroed on ScalarE, whose queue is free right after its
        # preamble (~6.3us) — gpsimd is busy with register init until ~7us
        # and the Sync/Pool DMA queues must not be delayed. This lets the
        # HAM pre-warm matmuls start at ~7.3us.
        wsrc = const.tile([128, CHUNK], F8, tag="wsrc")
        nc.scalar.memzero(wsrc[:])
        boot = const.tile([128, 512 + CHUNK], F8, tag="boot")
        nc.sync.dma_start(boot[:], boot_d[:])
        # xta rides the (idle) Pool engine's DMA queue, in parallel with the
        # Sync queue, so early row units land before the cold produces need them
        xta = const.tile([128, 2048], F8, tag="xta")
        nc.gpsimd.dma_start(xta[:], xta_d[:])
        schb = const.tile([128, 1], F32, tag="schb")
        nc.sync.dma_start(schb[:], schb_d[:])
        oha = const.tile([128, 8 * 2 * OHP], F8, tag="oha")
        nc.sync.dma_start(oha[:], oha_d[:])
        xtc1 = const.tile([128, CHUNK], F8, tag="xtc1")
        nc.sync.dma_start(xtc1[:], xtc1_d[:])
        ohb = const.tile([128, (N_PAIR - 8) * 2 * OHP], F8, tag="ohb")
        nc.sync.dma_start(ohb[:], ohb_d[:])
        xtb = const.tile([128, TWO_B - 512 - 2048], F8, tag="xtb")
        nc.sync.dma_start(xtb[:], xtb_d[:])
        cm = const.tile([NCLS + 1, COLS_PER_CORE], F16, tag="cm")
        nc.sync.dma_start(cm[:], cm_d[:])

        ohav = oha.rearrange("p (g two m) -> p g two m", two=2, m=OHP)
        ohbv = ohb.rearrange("p (g two m) -> p g two m", two=2, m=OHP)

        def ohv(g):
            return ohav[:, g] if g < 8 else ohbv[:, g - 8]

        def xtc_h(c):  # rhs for chunk c: this core's strip columns
            return boot[:, 512: 512 + CHUNK] if c == 0 else xtc1[:]

        def w1(u):  # produce lhsT for row unit u (0..63)
            if u < 4:
                return boot[:, ts(u, 128)]
            if u < 20:
                return xta[:, ts(u - 4, 128)]
            return xtb[:, ts(u - 20, 128)]

        # ones[0] = 0 so the stile reduction skips mk row 0, which carries
        # full (cmask row 0 is all-ones host-side) for a cheap SBUF copy.
        ones = const.tile([NCLS + 1, 1], F16, tag="ones")
        nc.gpsimd.memset(ones[:], 1.0)
        nc.gpsimd.memset(ones[0:1, :], 0.0)
        scratch = const.tile([1, 1], F32, tag="scratch")
        # trigger the exp table load during the DMA ramp, off the hot path
        warm = const.tile([1, 2], F32, tag="warm")
        nc.gpsimd.memset(warm[:], 0.0)
        nc.scalar.activation(warm[0:1, 1:2], warm[0:1, 0:1],
                             mybir.ActivationFunctionType.Exp, scale=1.0)
        # HAM pre-warm: the PE idles ~2us between its preamble and the boot
        # DMA landing. Garbage matmuls on wsrc (memset above) fill that
        # window so the 3.4us busy window that unthrottles the clock starts
        # at ~7.2us, not at the first real produce. Output goes to q0's
        # bank, which the first real consume's start=True wipes.

        def emit_produce_exp(c, g):
            z = zp.tile([128, 1024], F32, tag="z", name=f"z{c}_{g}")
            for s in range(2):
                u = 2 * g + s
                nc.tensor.matmul(
                    z[:, ts(s, CHUNK)], lhsT=w1(u), rhs=xtc_h(c),
                    start=True, stop=True, skip_group_check=True,
                )
            # dedicated ez buffer per pair: no pool rotation -> no WAR edges
            # back onto the PE consume stream -> no cross-engine wait cycles.
            ez = const.tile([128, 1024], F8, tag=f"ez{c}_{g}", name=f"ez{c}_{g}")
            if not ACT_PAT[g]:
                last_dve_ez[0] = ez
            if ACT_PAT[g]:
                nc.scalar.activation(
                    ez[:], z[:], mybir.ActivationFunctionType.Exp,
                    scale=1.0 / TEMPERATURE,
                )
            else:
                nc.vector.tensor_scalar(
                    ez.bitcast(I8)[:], z[:], SCH_A, schb[:, 0:1], MULT, ADD,
                )
            return ez

        def emit_consume(c, g, ez, q):
            ezv = ez.rearrange("p (two n) -> p two n", two=2)
            nc.tensor.matmul(
                q[0:OHW, :], lhsT=ohv(g)[:, :, 0:OHW], rhs=ezv[:],
                start=(g == 0), stop=(g == N_PAIR - 1),
                perf_mode=DR, skip_group_check=True,
            )

        fs2_d = fs_d.rearrange("a (h n) -> a h n", h=2)

        def emit_extract(c, q):
            mk = mkp.tile([NCLS + 1, CHUNK], F16, tag="mk", name=f"mk{c}")
            nc.vector.tensor_mul(mk[:], q[0: NCLS + 1, :], cm[:, ts(c, CHUNK)])
            # fire the full-row DMA before the stile matmul so it overlaps
            nc.sync.dma_start(fs2_d[0:1, 0, ds(c * CHUNK, CHUNK)], mk[0:1, :])
            stile = qp.tile([1, CHUNK], F32, tag="q", name=f"st{c}")
            nc.tensor.matmul(
                stile[0:1, :], lhsT=ones[:], rhs=mk[:],
                start=True, stop=True, skip_group_check=True,
            )
            sf = const.tile([1, CHUNK], F16, tag=f"sf{c}", name=f"sf{c}")
            nc.vector.tensor_copy(sf[:], stile[0:1, :])
            nc.sync.dma_start(fs2_d[0:1, 1, ds(c * CHUNK, CHUNK)], sf[:])

        # software pipeline: consume lags produce/exp by 3 pairs so the PE
        # never waits on the exp engines (which alternate ACT/DVE).
        # extract(0) is deferred several pairs past chunk-0's last consume so
        # its DVE copy (which waits on the PE) doesn't block queued DVE exps.
        from collections import deque

        last_dve_ez = [None]
        q0 = qp.tile([OHW, CHUNK], F32, tag="q", name="q0")
        for w in range(5):
            nc.tensor.matmul(
                q0[0:OHW, :], lhsT=wsrc[:, 0:OHW], rhs=wsrc[:],
                start=True, stop=True, skip_group_check=True,
            )
        q1 = qp.tile([OHW, CHUNK], F32, tag="q", name="q1")
        qs = {0: q0, 1: q1}
        sched = [(0, g) for g in range(N_PAIR)] + [(1, g) for g in range(N_PAIR)]
        LAG = 3
        pend = deque()
        done = 0
        for c, g in sched:
            pend.append((c, g, emit_produce_exp(c, g)))
            if len(pend) > LAG:
                cc, gg, ez = pend.popleft()
                emit_consume(cc, gg, ez, qs[cc])
                done += 1
            if done == N_PAIR + 6:  # several pairs after chunk-0 is consumed
                # absorb the cmask-DMA wait on a cheap DVE op HERE (cm has
                # long landed). The read of a late DVE-written ez pins this
                # op's schedule position (Tile otherwise hoists it early,
                # blocking the whole DVE stream on the cm DMA).
                nc.vector.tensor_mul(scratch[:], cm[0:1, 0:1],
                                     last_dve_ez[0][0:1, 0:1])
                emit_extract(0, q0)
                done += 1  # fire once
        while pend:
            cc, gg, ez = pend.popleft()
            emit_consume(cc, gg, ez, qs[cc])
        emit_extract(1, q1)

    _strip_self_engine_waits(nc)
    return nc


def _split_drain_waits(nc: bass.Bass, max_waits: int = 1) -> None:
    for bb in nc.main_func.blocks:
        out = []
        for ins in bb.instructions:
            si = ins.sync_info
            waits = list(si.on_wait) if si and si.on_wait else []
            if len(waits) > max_waits:
                chunks = [
                    waits[i: i + max_waits] for i in range(0, len(waits), max_waits)
                ]
                for j, ch in enumerate(chunks[:-1]):
                    out.append(
                        mybir.InstDrain(
                            name=f"{ins.name}-w{j}", ins=[], outs=[],
                            engine=ins.engine,
                            sync_info=mybir.SyncInfo(on_wait=ch, on_update=[]),
                        )
                    )
                ins.sync_info = mybir.SyncInfo(
                    on_wait=chunks[-1], on_update=list(si.on_update or [])
                )
            out.append(ins)
        bb.instructions[:] = out


def _strip_self_engine_waits(nc: bass.Bass) -> None:
    prefix = {
        mybir.EngineType.Activation: "Activation_",
        mybir.EngineType.PE: "PE_",
        mybir.EngineType.DVE: "DVE_",
        mybir.EngineType.Pool: "Pool_",
    }
    for bb in nc.main_func.blocks:
        for ins in bb.instructions:
            si = ins.sync_info
            if not si or not si.on_wait or len(si.on_wait) < 2:
                continue
            pref = prefix.get(ins.engine)
            if pref is None:
                continue
            kept = [w for w in si.on_wait if not (w.ant_name or "").startswith(pref)]
            if len(kept) != len(si.on_wait):
                ins.sync_info = mybir.SyncInfo(
                    on_wait=kept, on_update=list(si.on_update)
                )


def _get_program(split_waits: bool = True) -> bass.Bass:
    global _PROGRAM
    if _PROGRAM is None:
        _PROGRAM = _build_program()
        if split_waits:
            _split_drain_waits(_PROGRAM)
    return _PROGRAM


def _tune_schb(x8f):
    """Pick B minimizing |mean rel err| of the int8/fp8e4 Schraudolph exp
    over a sample of actual z values (fp8-quantized inputs)."""
    rng = np.random.default_rng(1)
    i = rng.integers(0, TWO_B, 4096)
    j = rng.integers(0, TWO_B, 4096)
    z = np.einsum("ij,ij->i", x8f[i], x8f[j])
    ref = np.exp(z / TEMPERATURE)
    best, bestb = 1e9, SCH_B_DEFAULT
    for b in np.arange(55.0, 56.6, 0.02):
        u = np.clip(np.rint(z * SCH_A + b), 1, 126).astype(np.uint8)
        import ml_dtypes
        val = u.view(np.int8).view(ml_dtypes.float8_e4m3).astype(np.float32)
        m = abs(np.mean(val / ref - 1))
        if m < best:
            best, bestb = m, b
    return float(bestb)


def _prepare_in_maps(out_1, out_2, target):
    import ml_dtypes

    x = np.concatenate(
        [np.asarray(out_1, np.float32), np.asarray(out_2, np.float32)], axis=0
    )
    x8 = x.astype(ml_dtypes.float8_e4m3)
    x8f = x8.astype(np.float32)
    xt = np.ascontiguousarray(x8.T)  # [128, 8192] fp8
    t2 = np.concatenate([np.asarray(target), np.asarray(target)]).astype(np.int64)

    schb = np.full((128, 1), _tune_schb(x8f), np.float32)

    # one-hot pair-packed [pair, 2, OHP] fp8: unit u rows 128u..128u+127
    oh = np.zeros((128, N_PAIR, 2, OHP), np.float32)
    for g in range(N_PAIR):
        for s2 in range(2):
            u = 2 * g + s2
            rows = t2[128 * u: 128 * (u + 1)]
            oh[:, g, s2, 0] = 1.0
            oh[np.arange(128), g, s2, 1 + rows] = 1.0
    oh8 = oh.reshape(128, N_PAIR * 2 * OHP).astype(ml_dtypes.float8_e4m3)

    in_maps = []
    for core in range(N_CORES):
        c0 = core * COLS_PER_CORE
        tcols = t2[c0: c0 + COLS_PER_CORE]
        cmask = (
            np.arange(NCLS + 1, dtype=np.int64)[:, None] == (1 + tcols)[None, :]
        ).astype(np.float16)
        cmask[0, :] = 1.0  # mk row 0 = q row 0 = full (stile skips it)
        boot = np.ascontiguousarray(
            np.concatenate([xt[:, 0:512], xt[:, c0: c0 + CHUNK]], axis=1)
        )
        in_maps.append(
            {
                "boot": boot,
                "xta": np.ascontiguousarray(xt[:, 512: 512 + 2048]),
                "oha": np.ascontiguousarray(oh8[:, : 8 * 2 * OHP]),
                "xtc1": np.ascontiguousarray(xt[:, c0 + CHUNK: c0 + COLS_PER_CORE]),
                "ohb": np.ascontiguousarray(oh8[:, 8 * 2 * OHP:]),
                "xtb": np.ascontiguousarray(xt[:, 512 + 2048:]),
                "cmask": cmask,
                "schb": schb,
            }
        )
    return in_maps


def _finish(fs_per_core) -> np.ndarray:
    full = np.concatenate(
        [np.asarray(f).reshape(-1)[:COLS_PER_CORE] for f in fs_per_core]
    ).astype(np.float64)
    s = np.concatenate(
        [np.asarray(f).reshape(-1)[COLS_PER_CORE:] for f in fs_per_core]
    ).astype(np.float64)
    n = TWO_B - 2
    ng = full - s
    o1 = full - (1.0 - TAU_PLUS) * ng
    o2 = full + (n * TAU_PLUS - (1.0 - TAU_PLUS)) * ng
    loss = float(np.mean(np.log(o2) - np.log(o1)))
    return np.array(loss, dtype=np.float32)


def run(out_1, out_2, out_m, target, trace=False):
    nc = _get_program()
    in_maps = _prepare_in_maps(out_1, out_2, target)
    # transient NRT_EXEC_UNIT_UNRECOVERABLE wedges resolve on retry
    last = None
    for attempt in range(3):
        try:
            res = run_bass_kernel_spmd(nc, in_maps, list(range(N_CORES)), trace=trace)
            break
        except Exception as e:
            last = e
    else:
        raise last
    fs = [res.results[i]["fs"] for i in range(N_CORES)]
    return _finish(fs), res.exec_time_ns


def kernel(out_1, out_2, out_m, target):
    loss, _ = run(out_1, out_2, out_m, target, trace=False)
    return loss


# revision 24
# speedup vs baseline: 1.0035x; 1.0035x over previous
"""DebiasedPosLossV2 on 8 NeuronCores — fp8 produce + dual-engine exp + fp8 DoubleRow.

Decomposition: each core owns a 1024-column strip of the 8192x8192 sim
matrix; produce matmuls build z = x @ x_strip^T 128 rows at a time, exp
runs split across ScalarE (activation) and DVE (Schraudolph int8 bit
trick), and a one-hot DoubleRow consume matmul reduces rows into
per-class sums (row 0 = all-ones = full). Host finishes the loss.

Changes vs the fp16 column-strip baseline (67.3us -> ~63us):
  * x is fp8e4 end-to-end: produce matmuls run at the same 1 col/cycle
    as fp16 (fp8 without a perf mode == bf16 speed on TRN2) but the xt
    DMA halves to 1 MB, so the PE pipeline starts ~1.7us after the DMA
    queue opens instead of ~6us in.
  * no dummy-matmul HAM warmup: real produces start as soon as boot
    (4 row units + chunk-0 rhs) lands and warm the PE clock themselves
    while the rest of xt streams; xta rides the Pool engine's DMA queue
    in parallel with the Sync queue.
  * cmask is fp16, fs output is f16 DMA'd straight from mk row 0 / a
    small stile copy (no staging tile, shorter extract tail).
  * exp split 18 ScalarE / 14 DVE per chunk (ScalarE is ~15% faster and
    DVE also carries the extract work).
"""

import sys

if "/opt/trn_rl_repo" not in sys.path:
    sys.path.insert(0, "/opt/trn_rl_repo")

from contextlib import ExitStack

import numpy as np

import concourse.bass as bass
import concourse.mybir as mybir
import concourse.tile as tile
from concourse.bass import ds, ts
from concourse.bass_utils import run_bass_kernel_spmd

B = 4096
D = 128
TWO_B = 2 * B
TEMPERATURE = 0.5
TAU_PLUS = 0.1
N_CORES = 8
COLS_PER_CORE = TWO_B // N_CORES  # 1024
CHUNK = 512
N_CHUNKS = COLS_PER_CORE // CHUNK  # 2
N_PAIR = TWO_B // 256             # 32 unit-pairs per chunk
NCLS = 100
OHW = 104                         # one-hot rows: 0 ones, 1..101 classes, pad
OHP = 112                         # padded pair stride (16B aligned)

F16 = mybir.dt.float16
F32 = mybir.dt.float32
F8 = mybir.dt.float8e4
I8 = mybir.dt.int8
MULT = mybir.AluOpType.mult
ADD = mybir.AluOpType.add
DR = mybir.MatmulPerfMode.DoubleRow

# Schraudolph fp8e4 constants: bits8(e^y) ~= y*8*log2(e) + (7*8 + c)
SCH_A = (1.0 / TEMPERATURE) * 8.0 * 1.4426950408889634
SCH_B_DEFAULT = 56.0 - 0.46

# 18 of 32 pairs on ScalarE, 14 on DVE — ScalarE exp is ~15% faster and the
# DVE also carries the extract work
ACT_PAT = [((g + 1) * 18) // 32 != (g * 18) // 32 for g in range(32)]
# the ramp puts pairs 30+31 both on ScalarE — their exps then serialize in
# the pipeline drain. Swap 29<->30 so the final two exps run on different
# engines concurrently.
ACT_PAT[29], ACT_PAT[30] = True, False

_PROGRAM = None


def _build_program() -> bass.Bass:
    nc = bass.Bass()

    # DMA order tuned so the pipeline never starves: boot first (4 xt row
    # units + chunk-0 rhs), xta in parallel on the Pool queue, then schb /
    # one-hots / the rest of the xt stream ahead of their consumers.
    boot_d = nc.declare_dram_parameter("boot", [128, 512 + CHUNK], F8, isOutput=False)
    xta_d = nc.declare_dram_parameter("xta", [128, 2048], F8, isOutput=False)
    oha_d = nc.declare_dram_parameter("oha", [128, 8 * 2 * OHP], F8, isOutput=False)
    xtc1_d = nc.declare_dram_parameter("xtc1", [128, CHUNK], F8, isOutput=False)
    ohb_d = nc.declare_dram_parameter("ohb", [128, (N_PAIR - 8) * 2 * OHP], F8, isOutput=False)
    xtb_d = nc.declare_dram_parameter("xtb", [128, TWO_B - 512 - 2048], F8, isOutput=False)
    cm_d = nc.declare_dram_parameter("cmask", [NCLS + 1, COLS_PER_CORE], F16, isOutput=False)
    schb_d = nc.declare_dram_parameter("schb", [128, 1], F32, isOutput=False)
    fs_d = nc.declare_dram_parameter("fs", [1, 2 * COLS_PER_CORE], F16, isOutput=True)

    with ExitStack() as ctx:
        tc = ctx.enter_context(tile.TileContext(nc))
        const = ctx.enter_context(tc.tile_pool(name="const", bufs=1))
        mkp = ctx.enter_context(tc.tile_pool(name="mk", bufs=2))
        zp = ctx.enter_context(tc.tile_pool(name="z", bufs=3, space="PSUM"))
        qp = ctx.enter_context(tc.tile_pool(name="q", bufs=2, space="PSUM"))

        # wsrc memset first in the gpsimd queue (right after the Bass-init
        # canaries, before the xta SWDGE descriptor generation) so the HAM
        # pre-warm matmuls can start as soon as the PE queue opens
        wsrc = const.tile([128, CHUNK], F8, tag="wsrc")
        nc.gpsimd.memset(wsrc[:], 0.0)
        boot = const.tile([128, 512 + CHUNK], F8, tag="boot")
        nc.sync.dma_start(boot[:], boot_d[:])
        # xta rides the (idle) Pool engine's DMA queue, in parallel with the
        # Sync queue, so early row units land before the cold produces need them
        xta = const.tile([128, 2048], F8, tag="xta")
        nc.gpsimd.dma_start(xta[:], xta_d[:])
        schb = const.tile([128, 1], F32, tag="schb")
        nc.sync.dma_start(schb[:], schb_d[:])
        oha = const.tile([128, 8 * 2 * OHP], F8, tag="oha")
        nc.sync.dma_start(oha[:], oha_d[:])
        xtc1 = const.tile([128, CHUNK], F8, tag="xtc1")
        nc.sync.dma_start(xtc1[:], xtc1_d[:])
        ohb = const.tile([128, (N_PAIR - 8) * 2 * OHP], F8, tag="ohb")
        nc.sync.dma_start(ohb[:], ohb_d[:])
        xtb = const.tile([128, TWO_B - 512 - 2048], F8, tag="xtb")
        nc.sync.dma_start(xtb[:], xtb_d[:])
        cm = const.tile([NCLS + 1, COLS_PER_CORE], F16, tag="cm")
        nc.sync.dma_start(cm[:], cm_d[:])

        ohav = oha.rearrange("p (g two m) -> p g two m", two=2, m=OHP)
        ohbv = ohb.rearrange("p (g two m) -> p g two m", two=2, m=OHP)

        def ohv(g):
            return ohav[:, g] if g < 8 else ohbv[:, g - 8]

        def xtc_h(c):  # rhs for chunk c: this core's strip columns
            return boot[:, 512: 512 + CHUNK] if c == 0 else xtc1[:]

        def w1(u):  # produce lhsT for row unit u (0..63)
            if u < 4:
                return boot[:, ts(u, 128)]
            if u < 20:
                return xta[:, ts(u - 4, 128)]
            return xtb[:, ts(u - 20, 128)]

        # ones[0] = 0 so the stile reduction skips mk row 0, which carries
        # full (cmask row 0 is all-ones host-side) for a cheap SBUF copy.
        ones = const.tile([NCLS + 1, 1], F16, tag="ones")
        nc.gpsimd.memset(ones[:], 1.0)
        nc.gpsimd.memset(ones[0:1, :], 0.0)
        scratch = const.tile([1, 1], F32, tag="scratch")
        # trigger the exp table load during the DMA ramp, off the hot path
        warm = const.tile([1, 2], F32, tag="warm")
        nc.gpsimd.memset(warm[:], 0.0)
        nc.scalar.activation(warm[0:1, 1:2], warm[0:1, 0:1],
                             mybir.ActivationFunctionType.Exp, scale=1.0)
        # HAM pre-warm: the PE idles ~2us between its preamble and the boot
        # DMA landing. Garbage matmuls on wsrc (memset above) fill that
        # window so the 3.4us busy window that unthrottles the clock starts
        # at ~7.2us, not at the first real produce. Output goes to q0's
        # bank, which the first real consume's start=True wipes.

        def emit_produce_exp(c, g):
            z = zp.tile([128, 1024], F32, tag="z", name=f"z{c}_{g}")
            for s in range(2):
                u = 2 * g + s
                nc.tensor.matmul(
                    z[:, ts(s, CHUNK)], lhsT=w1(u), rhs=xtc_h(c),
                    start=True, stop=True, skip_group_check=True,
                )
            # dedicated ez buffer per pair: no pool rotation -> no WAR edges
            # back onto the PE consume stream -> no cross-engine wait cycles.
            ez = const.tile([128, 1024], F8, tag=f"ez{c}_{g}", name=f"ez{c}_{g}")
            if not ACT_PAT[g]:
                last_dve_ez[0] = ez
            if ACT_PAT[g]:
                nc.scalar.activation(
                    ez[:], z[:], mybir.ActivationFunctionType.Exp,
                    scale=1.0 / TEMPERATURE,
                )
            else:
                nc.vector.tensor_scalar(
                    ez.bitcast(I8)[:], z[:], SCH_A, schb[:, 0:1], MULT, ADD,
                )
            return ez

        def emit_consume(c, g, ez, q):
            ezv = ez.rearrange("p (two n) -> p two n", two=2)
            nc.tensor.matmul(
                q[0:OHW, :], lhsT=ohv(g)[:, :, 0:OHW], rhs=ezv[:],
                start=(g == 0), stop=(g == N_PAIR - 1),
                perf_mode=DR, skip_group_check=True,
            )

        fs2_d = fs_d.rearrange("a (h n) -> a h n", h=2)

        def emit_extract(c, q):
            # mk-mul and stile matmul run in 256-col halves so the PE's
            # stile work overlaps the DVE's second half-multiply (shorter
            # serial tail after the last consume)
            mk = mkp.tile([NCLS + 1, CHUNK], F16, tag="mk", name=f"mk{c}")
            stile = qp.tile([1, CHUNK], F32, tag="q", name=f"st{c}")
            for h in range(2):
                hs = ds(h * (CHUNK // 2), CHUNK // 2)
                nc.vector.tensor_mul(mk[:, hs], q[0: NCLS + 1, hs],
                                     cm[:, c * CHUNK + h * (CHUNK // 2):
                                         c * CHUNK + (h + 1) * (CHUNK // 2)])
                nc.tensor.matmul(
                    stile[0:1, hs], lhsT=ones[:], rhs=mk[:, hs],
                    start=True, stop=True, skip_group_check=True,
                )
            # fire the full-row DMA as soon as mk is complete
            nc.sync.dma_start(fs2_d[0:1, 0, ds(c * CHUNK, CHUNK)], mk[0:1, :])
            sf = const.tile([1, CHUNK], F16, tag=f"sf{c}", name=f"sf{c}")
            nc.vector.tensor_copy(sf[:], stile[0:1, :])
            nc.sync.dma_start(fs2_d[0:1, 1, ds(c * CHUNK, CHUNK)], sf[:])

        # software pipeline: consume lags produce/exp by 3 pairs so the PE
        # never waits on the exp engines (which alternate ACT/DVE).
        # extract(0) is deferred several pairs past chunk-0's last consume so
        # its DVE copy (which waits on the PE) doesn't block queued DVE exps.
        from collections import deque

        last_dve_ez = [None]
        q0 = qp.tile([OHW, CHUNK], F32, tag="q", name="q0")
        for w in range(5):
            nc.tensor.matmul(
                q0[0:OHW, :], lhsT=wsrc[:, 0:OHW], rhs=wsrc[:],
                start=True, stop=True, skip_group_check=True,
            )
        q1 = qp.tile([OHW, CHUNK], F32, tag="q", name="q1")
        qs = {0: q0, 1: q1}
        sched = [(0, g) for g in range(N_PAIR)] + [(1, g) for g in range(N_PAIR)]
        LAG = 3
        pend = deque()
        done = 0
        for c, g in sched:
            pend.append((c, g, emit_produce_exp(c, g)))
            if len(pend) > LAG:
                cc, gg, ez = pend.popleft()
                emit_consume(cc, gg, ez, qs[cc])
                done += 1
            if done == N_PAIR + 6:  # several pairs after chunk-0 is consumed
                # absorb the cmask-DMA wait on a cheap DVE op HERE (cm has
                # long landed). The read of a late DVE-written ez pins this
                # op's schedule position (Tile otherwise hoists it early,
                # blocking the whole DVE stream on the cm DMA).
                nc.vector.tensor_mul(scratch[:], cm[0:1, 0:1],
                                     last_dve_ez[0][0:1, 0:1])
                emit_extract(0, q0)
                done += 1  # fire once
        while pend:
            cc, gg, ez = pend.popleft()
            emit_consume(cc, gg, ez, qs[cc])
        emit_extract(1, q1)

    _strip_self_engine_waits(nc)
    return nc


def _split_drain_waits(nc: bass.Bass, max_waits: int = 1) -> None:
    for bb in nc.main_func.blocks:
        out = []
        for ins in bb.instructions:
            si = ins.sync_info
            waits = list(si.on_wait) if si and si.on_wait else []
            if len(waits) > max_waits:
                chunks = [
                    waits[i: i + max_waits] for i in range(0, len(waits), max_waits)
                ]
                for j, ch in enumerate(chunks[:-1]):
                    out.append(
                        mybir.InstDrain(
                            name=f"{ins.name}-w{j}", ins=[], outs=[],
                            engine=ins.engine,
                            sync_info=mybir.SyncInfo(on_wait=ch, on_update=[]),
                        )
                    )
                ins.sync_info = mybir.SyncInfo(
                    on_wait=chunks[-1], on_update=list(si.on_update or [])
                )
            out.append(ins)
        bb.instructions[:] = out


def _strip_self_engine_waits(nc: bass.Bass) -> None:
    prefix = {
        mybir.EngineType.Activation: "Activation_",
        mybir.EngineType.PE: "PE_",
        mybir.EngineType.DVE: "DVE_",
        mybir.EngineType.Pool: "Pool_",
    }
    for bb in nc.main_func.blocks:
        for ins in bb.instructions:
            si = ins.sync_info
            if not si or not si.on_wait or len(si.on_wait) < 2:
                continue
            pref = prefix.get(ins.engine)
            if pref is None:
                continue
            kept = [w for w in si.on_wait if not (w.ant_name or "").startswith(pref)]
            if len(kept) != len(si.on_wait):
                ins.sync_info = mybir.SyncInfo(
                    on_wait=kept, on_update=list(si.on_update)
                )


def _get_program(split_waits: bool = True) -> bass.Bass:
    global _PROGRAM
    if _PROGRAM is None:
        _PROGRAM = _build_program()
        if split_waits:
            _split_drain_waits(_PROGRAM)
    return _PROGRAM


def _tune_schb(x8f):
    """Pick B minimizing |mean rel err| of the int8/fp8e4 Schraudolph exp
    over a sample of actual z values (fp8-quantized inputs)."""
    rng = np.random.default_rng(1)
    i = rng.integers(0, TWO_B, 4096)
    j = rng.integers(0, TWO_B, 4096)
    z = np.einsum("ij,ij->i", x8f[i], x8f[j])
    ref = np.exp(z / TEMPERATURE)
    best, bestb = 1e9, SCH_B_DEFAULT
    for b in np.arange(55.0, 56.6, 0.02):
        u = np.clip(np.rint(z * SCH_A + b), 1, 126).astype(np.uint8)
        import ml_dtypes
        val = u.view(np.int8).view(ml_dtypes.float8_e4m3).astype(np.float32)
        m = abs(np.mean(val / ref - 1))
        if m < best:
            best, bestb = m, b
    return float(bestb)


def _prepare_in_maps(out_1, out_2, target):
    import ml_dtypes

    x = np.concatenate(
        [np.asarray(out_1, np.float32), np.asarray(out_2, np.float32)], axis=0
    )
    x8 = x.astype(ml_dtypes.float8_e4m3)
    x8f = x8.astype(np.float32)
    xt = np.ascontiguousarray(x8.T)  # [128, 8192] fp8
    t2 = np.concatenate([np.asarray(target), np.asarray(target)]).astype(np.int64)

    schb = np.full((128, 1), _tune_schb(x8f), np.float32)

    # one-hot pair-packed [pair, 2, OHP] fp8: unit u rows 128u..128u+127
    oh = np.zeros((128, N_PAIR, 2, OHP), np.float32)
    for g in range(N_PAIR):
        for s2 in range(2):
            u = 2 * g + s2
            rows = t2[128 * u: 128 * (u + 1)]
            oh[:, g, s2, 0] = 1.0
            oh[np.arange(128), g, s2, 1 + rows] = 1.0
    oh8 = oh.reshape(128, N_PAIR * 2 * OHP).astype(ml_dtypes.float8_e4m3)

    in_maps = []
    for core in range(N_CORES):
        c0 = core * COLS_PER_CORE
        tcols = t2[c0: c0 + COLS_PER_CORE]
        cmask = (
            np.arange(NCLS + 1, dtype=np.int64)[:, None] == (1 + tcols)[None, :]
        ).astype(np.float16)
        cmask[0, :] = 1.0  # mk row 0 = q row 0 = full (stile skips it)
        boot = np.ascontiguousarray(
            np.concatenate([xt[:, 0:512], xt[:, c0: c0 + CHUNK]], axis=1)
        )
        in_maps.append(
            {
                "boot": boot,
                "xta": np.ascontiguousarray(xt[:, 512: 512 + 2048]),
                "oha": np.ascontiguousarray(oh8[:, : 8 * 2 * OHP]),
                "xtc1": np.ascontiguousarray(xt[:, c0 + CHUNK: c0 + COLS_PER_CORE]),
                "ohb": np.ascontiguousarray(oh8[:, 8 * 2 * OHP:]),
                "xtb": np.ascontiguousarray(xt[:, 512 + 2048:]),
                "cmask": cmask,
                "schb": schb,
            }
        )
    return in_maps


def _finish(fs_per_core) -> np.ndarray:
    full = np.concatenate(
        [np.asarray(f).reshape(-1)[:COLS_PER_CORE] for f in fs_per_core]
    ).astype(np.float64)
    s = np.concatenate(
        [np.asarray(f).reshape(-1)[COLS_PER_CORE:] for f in fs_per_core]
    ).astype(np.float64)
    n = TWO_B - 2
    ng = full - s
    o1 = full - (1.0 - TAU_PLUS) * ng
    o2 = full + (n * TAU_PLUS - (1.0 - TAU_PLUS)) * ng
    loss = float(np.mean(np.log(o2) - np.log(o1)))
    return np.array(loss, dtype=np.float32)


def run(out_1, out_2, out_m, target, trace=False):
    nc = _get_program()
    in_maps = _prepare_in_maps(out_1, out_2, target)
    # transient NRT_EXEC_UNIT_UNRECOVERABLE wedges resolve on retry
    last = None
    for attempt in range(3):
        try:
            res = run_bass_kernel_spmd(nc, in_maps, list(range(N_CORES)), trace=trace)
            break
        except Exception as e:
            last = e
    else:
        raise last
    fs = [res.results[i]["fs"] for i in range(N_CORES)]
    return _finish(fs), res.exec_time_ns


def kernel(out_1, out_2, out_m, target):
    loss, _ = run(out_1, out_2, out_m, target, trace=False)
    return loss
